# revision 39
# baseline (speedup 1.0000x reference)
"""KMultiHeadedAttention Trainium2 kernel (v2).

Full-input contract: kernel(**inputs) takes the unsharded numpy inputs and
returns the full [4, 256, 2048] output. Internally shards (batch, N-half)
across 8 NeuronCores; each core computes QKV projections for its batch, the
attention for its 1024 query positions, and the output projection for its
slice. Output slices are disjoint -> plain concatenation, no collectives.

Per-core pipeline (b = batch, n in its half; head-major channel perm p):
  host: xk = key[b] * weight[b]   (the multiplicative key-wise weight
        commutes through the 1x1 conv: (Wk@key)*w = Wk@(key*w); a nonzero
        bk is restored as the rank-1 update bk (x) w on the PE)
  q = (Wq[p]/8) @ xq   [256, 1024]  -- PSUM -> SBUF as f32r copies split
  k = Wk[p] @ xk       [256, 2048]  -- across DVE/Act (f32r: 1 PE cyc/row)
  vTa[m, 65h+d] = xv^T @ Wv[p]^T with a ones column per head (rank-1
        ones x bva matmul), converted f32->bf16 on Act
  ST[m, n] = k_h^T q_h            f32r matmuls into [128,1024] PSUM pairs
  sm = ST * maskT                 DVE 1024-wide (the ONLY psum-reading
                                  elementwise pass; mask bf16, broadcast
                                  across the 2 heads sharing a stg tile)
  E = exp(sm)                     Act 4096-wide bf16 tiles (mc-pairs)
  Xa[n, 65h:65h+65] += E_slice^T @ vTa_slice   (PV transposed: 65-row
        output free dim halves PE rows vs the [d, n] layout). NOTE:
        matmul start=True zeroes the WHOLE psum bank, so only the first
        accumulation group per xa tile sets it.
  x[n, c] = Xa[:, :64] * recip(Xa[:, 64])  (strided recip on DVE; scaled
        Copy on Act / tensor_scalar on DVE)
  xT via SBUF->SBUF DmaTranspose (j-major layout), out = Wm[p'] @ xT

Schedule: two n-512 groups. g0 staging overlaps the v-projection; g1
staging carries PV(g0) as mc-pairs in slots 0-7, xdiv(g0) in slots 8-11,
and pre-starts PV(g1) pairs 0-4 in slots 11-15 so the tail is only the
last PV pairs + xdiv(g1) + transposes + output projection, with the tail
elementwise on whichever of DVE/Act is idle.

TimelineSim: 104.6 us/core (baseline 141.9).
"""

import sys

sys.path.insert(0, "/opt/trn_rl_repo")

import ml_dtypes
import numpy as np

B, D, N, M = 4, 256, 2048, 2048
H, HD = 4, 64
NCORES = 8
NH = N // 2  # query positions per core
AUG = HD + 1  # per-head vTa columns incl. ones column
VA = H * AUG  # 260
MCH = M // 128  # 16 m-chunks
BF = ml_dtypes.bfloat16

# new channel c' = 64h + d  <-  old channel 4d + h
_PERM = np.array([4 * d + h for h in range(H) for d in range(HD)])

_NC = None
_NC_KEY = None


def _build_nc(has_bq=False, has_bk=False, has_bm=False):
    import concourse.bass as bass
    import concourse.tile as tile
    from concourse import mybir

    f32 = mybir.dt.float32
    f32r = mybir.dt.float32r
    bf16 = mybir.dt.bfloat16
    Alu = mybir.AluOpType
    Act = mybir.ActivationFunctionType

    nc = bass.Bass()
    xq_d = nc.declare_dram_parameter("xq", [D, NH], bf16, isOutput=False)
    xk_d = nc.declare_dram_parameter("xk", [D, M], bf16, isOutput=False)
    xv_d = nc.declare_dram_parameter("xv", [D, M], bf16, isOutput=False)
    maskT_d = nc.declare_dram_parameter("maskT", [M, NH], bf16, isOutput=False)
    wq_d = nc.declare_dram_parameter("wqT", [D, D], bf16, isOutput=False)
    wk_d = nc.declare_dram_parameter("wkT", [D, D], bf16, isOutput=False)
    wv_d = nc.declare_dram_parameter("wvT", [D, VA], bf16, isOutput=False)
    wm_d = nc.declare_dram_parameter("wmT", [D, D], bf16, isOutput=False)
    bva_d = nc.declare_dram_parameter("bvar", [1, VA], bf16, isOutput=False)
    if has_bq:
        bqr_d = nc.declare_dram_parameter("bqr", [1, D], bf16, isOutput=False)
    if has_bk:
        bkr_d = nc.declare_dram_parameter("bkr", [1, D], bf16, isOutput=False)
        wrow_d = nc.declare_dram_parameter("wrow", [1, M], bf16, isOutput=False)
    if has_bm:
        bmr_d = nc.declare_dram_parameter("bmr", [1, D], bf16, isOutput=False)
    out_d = nc.declare_dram_parameter("out", [D, NH], f32, isOutput=True)

    with tile.TileContext(nc) as tc:
        with (
            tc.tile_pool(name="consts", bufs=1) as consts,
            tc.tile_pool(name="pin", bufs=1) as pin,
            tc.tile_pool(name="persist", bufs=1) as persist,
            tc.tile_pool(name="work", bufs=3) as work,
            tc.tile_pool(name="ps", bufs=2, space="PSUM") as psum,
        ):
            # ---- weights + activations (DMAs; tile deps gate consumers) ----
            wq_sb, wk_sb, wv_sb, wm_sb = [], [], [], []
            xq_sb, xk_sb, xv_sb = [], [], []
            for i in range(2):
                t = consts.tile([128, D], bf16, tag=f"wk{i}", name=f"wk{i}")
                nc.sync.dma_start(out=t, in_=wk_d[i * 128 : (i + 1) * 128, :])
                wk_sb.append(t)
                xk_sb.append(pin.tile([128, M], bf16, tag=f"xk{i}", name=f"xk{i}"))
                nc.sync.dma_start(
                    out=xk_sb[i][:, 0:1024], in_=xk_d[i * 128 : (i + 1) * 128, 0:1024]
                )
            for i in range(2):
                t = consts.tile([128, D], bf16, tag=f"wq{i}", name=f"wq{i}")
                nc.sync.dma_start(out=t, in_=wq_d[i * 128 : (i + 1) * 128, :])
                wq_sb.append(t)
                t = pin.tile([128, NH], bf16, tag=f"xq{i}", name=f"xq{i}")
                nc.sync.dma_start(out=t, in_=xq_d[i * 128 : (i + 1) * 128, :])
                xq_sb.append(t)
            for i in range(2):
                nc.sync.dma_start(
                    out=xk_sb[i][:, 1024:2048],
                    in_=xk_d[i * 128 : (i + 1) * 128, 1024:2048],
                )
            if has_bq:
                bqr_sb = consts.tile([1, D], bf16, tag="bqr", name="bqr")
                nc.sync.dma_start(out=bqr_sb, in_=bqr_d[:])
            if has_bk:
                bkr_sb = consts.tile([1, D], bf16, tag="bkr", name="bkr")
                nc.sync.dma_start(out=bkr_sb, in_=bkr_d[:])
                wrow_sb = consts.tile([1, M], bf16, tag="wrow", name="wrow")
                nc.sync.dma_start(out=wrow_sb, in_=wrow_d[:])
            if has_bm:
                bmr_sb = consts.tile([1, D], bf16, tag="bmr", name="bmr")
                nc.sync.dma_start(out=bmr_sb, in_=bmr_d[:])
            ones1 = consts.tile([1, 512], bf16, tag="ones1", name="ones1")
            nc.vector.memset(ones1, 1.0)

            # ---- projections: k(mh0) first so STs can start early ----
            q_sb = [persist.tile([128, NH], f32r, tag=f"q{i}", name=f"qsb{i}") for i in range(2)]
            k_sb = [persist.tile([128, M], f32r, tag=f"k{i}", name=f"ksb{i}") for i in range(2)]

            def emit_kproj(oc, mh, eng):
                if mh == 0:
                    pss = [psum.tile([128, 1024], f32, tag="stg", name=f"kp{oc}{mh}")]
                    slices = [(pss[0][:, j : j + 512], j) for j in (0, 512)]
                else:
                    pss = [
                        psum.tile([128, 512], f32, tag="xa", bufs=4, name=f"kp{oc}{mh}{j}")
                        for j in (0, 512)
                    ]
                    slices = [(pss[0], 0), (pss[1], 512)]
                for ps, j in slices:
                    for ic in range(2):
                        nc.tensor.matmul(
                            ps,
                            lhsT=wk_sb[ic][:, oc * 128 : (oc + 1) * 128],
                            rhs=xk_sb[ic][:, mh * 1024 + j : mh * 1024 + j + 512],
                            start=(ic == 0),
                            stop=(ic == 1) and not has_bk,
                        )
                    if has_bk:
                        # (Wk@key + bk)*w = Wk@(key*w) + bk (x) w
                        nc.tensor.matmul(
                            ps,
                            lhsT=bkr_sb[:, oc * 128 : (oc + 1) * 128],
                            rhs=wrow_sb[:, mh * 1024 + j : mh * 1024 + j + 512],
                            start=False,
                            stop=True,
                        )
                    dst = k_sb[oc][:, mh * 1024 + j : mh * 1024 + j + 512]
                    e = eng if eng != "mix" else ("dve" if oc == 0 else "act")
                    if e == "dve":
                        nc.vector.tensor_copy(out=dst, in_=ps)
                    else:
                        nc.scalar.activation(out=dst, in_=ps, func=Act.Copy)

            for oc in range(2):
                emit_kproj(oc, 0, "mix")
            for oc in range(2):
                for j in (0, 512):
                    ps = psum.tile([128, 512], f32, tag="xa", bufs=4, name=f"qp{oc}{j}")
                    for ic in range(2):
                        nc.tensor.matmul(
                            ps,
                            lhsT=wq_sb[ic][:, oc * 128 : (oc + 1) * 128],
                            rhs=xq_sb[ic][:, j : j + 512],
                            start=(ic == 0),
                            stop=(ic == 1) and not has_bq,
                        )
                    if has_bq:
                        nc.tensor.matmul(
                            ps,
                            lhsT=bqr_sb[:, oc * 128 : (oc + 1) * 128],
                            rhs=ones1,
                            start=False,
                            stop=True,
                        )
                    dst = q_sb[oc][:, j : j + 512]
                    if oc == 0:
                        nc.scalar.activation(out=dst, in_=ps, func=Act.Copy)
                    else:
                        nc.vector.tensor_copy(out=dst, in_=ps)
            for oc in range(2):
                emit_kproj(oc, 1, "act")

            # first few mask tiles before the v/w loads (the first mults
            # need them early); the rest prefetch during staging
            msk_sb = []
            for mc in range(MCH):
                msk_sb.append(
                    persist.tile([128, NH], bf16, tag=f"mk{mc}", name=f"mk{mc}")
                )
            for mc in range(6):
                nc.sync.dma_start(
                    out=msk_sb[mc], in_=maskT_d[mc * 128 : (mc + 1) * 128, :]
                )
            for i in range(2):
                t = pin.tile([128, M], bf16, tag=f"xv{i}", name=f"xv{i}")
                nc.sync.dma_start(out=t, in_=xv_d[i * 128 : (i + 1) * 128, :])
                xv_sb.append(t)
                t = consts.tile([128, VA], bf16, tag=f"wv{i}", name=f"wv{i}")
                nc.sync.dma_start(out=t, in_=wv_d[i * 128 : (i + 1) * 128, :])
                wv_sb.append(t)
                t = consts.tile([128, D], bf16, tag=f"wm{i}", name=f"wm{i}")
                nc.sync.dma_start(out=t, in_=wm_d[i * 128 : (i + 1) * 128, :])
                wm_sb.append(t)
            bva_sb = consts.tile([1, VA], bf16, tag="bvar", name="bvar")
            nc.sync.dma_start(out=bva_sb, in_=bva_d[:])
            for mc in range(6, MCH):
                nc.sync.dma_start(
                    out=msk_sb[mc], in_=maskT_d[mc * 128 : (mc + 1) * 128, :]
                )

            vta_sb = [
                persist.tile([128, VA], bf16, tag=f"vta{mc}", name=f"vta{mc}")
                for mc in range(MCH)
            ]
            # E tiles hold TWO m-chunks ([mc even | mc odd]) so exp runs on
            # 4096-wide tiles (fewer Act access-latency prologues)
            e_sb = [
                persist.tile([128, 4096], bf16, tag=f"e{p}", name=f"e{p}")
                for p in range(MCH // 2)
            ]
            # x in [n, c] layout: per n-chunk-of-128 a [128, 256] tile holding
            # all 4 heads' 64 channels side by side (transpose-DMA input)
            xp_sb = [
                persist.tile([128, 2 * D], bf16, tag=f"xp{jp}", name=f"xp{jp}")
                for jp in range(4)
            ]
            # xT: [c', n] j-major: xT[p, j*256 + a*128 + n] = x channel
            # a*128+p at position j*128+n  (lets one DmaTranspose cover a
            # j-PAIR and keeps the o-proj rhs a simple strided AP)
            xT_sb = persist.tile([128, 2 * NH], bf16, tag="xT", name="xT")

            sm_pair = [None]

            def emit_st_slot(g, mc):
                """ST matmuls + mask mult for slot (g, mc); exp fires on the
                4096-wide sm pair after each odd mc."""
                stg0 = psum.tile([128, 1024], f32, tag="stg", name=f"st{g}_{mc}a")
                stg1 = psum.tile([128, 1024], f32, tag="stg", name=f"st{g}_{mc}b")
                for h in range(H):
                    t = stg0 if h < 2 else stg1
                    po = 64 * (h % 2)
                    nc.tensor.matmul(
                        t[:, 512 * (h % 2) : 512 * (h % 2) + 512],
                        lhsT=k_sb[h // 2][po : po + 64, mc * 128 : (mc + 1) * 128],
                        rhs=q_sb[h // 2][po : po + 64, g * 512 : g * 512 + 512],
                        start=True,
                        stop=True,
                    )
                if mc % 2 == 0:
                    sm_pair[0] = work.tile(
                        [128, 4096], bf16, tag="sm", bufs=3, name=f"sm{g}_{mc}"
                    )
                sm = sm_pair[0]
                off = 2048 * (mc % 2)
                mk = msk_sb[mc][:, g * 512 : g * 512 + 512]
                mkb = mk.unsqueeze(1).to_broadcast([128, 2, 512])
                for hp, stg in ((0, stg0), (1, stg1)):
                    nc.vector.tensor_tensor(
                        out=sm[:, off + 1024 * hp : off + 1024 * hp + 1024],
                        in0=stg,
                        in1=mkb,
                        op=Alu.mult,
                    )
                if g == 1 and mc == 15:
                    for hp in range(2):
                        nc.scalar.activation(
                            out=e_sb[7][:, off + 1024 * hp : off + 1024 * hp + 1024],
                            in_=sm[:, off + 1024 * hp : off + 1024 * hp + 1024],
                            func=Act.Exp,
                        )
                elif g == 1 and mc == 14:
                    nc.scalar.activation(
                        out=e_sb[7][:, off : off + 2048],
                        in_=sm[:, off : off + 2048],
                        func=Act.Exp,
                    )
                elif mc % 2 == 1:
                    nc.scalar.activation(out=e_sb[mc // 2], in_=sm, func=Act.Exp)

            def emit_v_piece(mc):
                vps0 = psum.tile([128, 512], f32, tag="xa", bufs=4, name=f"vp{mc}")
                vps = vps0[:, 0:VA]
                for ic in range(2):
                    nc.tensor.matmul(
                        vps,
                        lhsT=xv_sb[ic][:, mc * 128 : (mc + 1) * 128],
                        rhs=wv_sb[ic],
                        start=(ic == 0),
                        stop=False,
                    )
                # ones column (and bv bias if nonzero) as rank-1 ones x bva
                nc.tensor.matmul(
                    vps, lhsT=ones1[:, 0:128], rhs=bva_sb, start=False, stop=True
                )
                nc.scalar.activation(out=vta_sb[mc], in_=vps, func=Act.Copy)

            def emit_pv_slot(xa, mc):
                # NOTE: start=True zeroes the WHOLE psum bank, so only the
                # first matmul of each xa tile may set it; the other head
                # groups accumulate from the zeroed bank.
                for j in range(4):
                    for h in range(H):
                        nc.tensor.matmul(
                            xa[j][:, AUG * h : AUG * h + AUG],
                            lhsT=e_sb[mc // 2][
                                :,
                                2048 * (mc % 2) + 512 * h + 128 * j : 2048 * (mc % 2)
                                + 512 * h
                                + 128 * (j + 1),
                            ],
                            rhs=vta_sb[mc][:, AUG * h : AUG * h + AUG],
                            start=(mc == 0 and h == 0),
                            stop=(mc == MCH - 1),
                            skip_group_check=True,
                        )

            def emit_xdiv_j(xa, g, j, eng="act"):
                rr = work.tile([128, H], f32, tag="rr", bufs=4, name=f"rr{g}_{j}")
                den = (
                    xa[j][:, HD : HD + AUG * H]
                    .rearrange("p (a b) -> p a b", b=AUG)[:, :, 0]
                )
                nc.vector.reciprocal(out=rr, in_=den)
                jj = 4 * g + j
                xp = xp_sb[jj // 2][:, 256 * (jj % 2) : 256 * (jj % 2) + 256]
                for h in range(H):
                    if eng == "act":
                        nc.scalar.activation(
                            out=xp[:, HD * h : HD * h + HD],
                            in_=xa[j][:, AUG * h : AUG * h + HD],
                            func=Act.Copy,
                            scale=rr[:, h : h + 1],
                        )
                    else:
                        nc.vector.tensor_scalar(
                            out=xp[:, HD * h : HD * h + HD],
                            in0=xa[j][:, AUG * h : AUG * h + HD],
                            scalar1=rr[:, h : h + 1],
                            scalar2=None,
                            op0=Alu.mult,
                        )
                if jj % 2 == 1:
                    # transpose the pair [n128, 512] -> 4 xT column blocks
                    dst = xT_sb[:, (jj - 1) * 256 : (jj + 1) * 256].rearrange(
                        "p (a n) -> p a n", a=4
                    )
                    nc.sync.dma_start_transpose(out=dst, in_=xp_sb[jj // 2][:])

            # ---- attention ----
            # g=0: staging + v-projection pieces
            for mc in range(MCH):
                emit_st_slot(0, mc)
                emit_v_piece(mc)
            # g=1: staging; PV(g=0) runs as mc-PAIRS in slots 0-7 so that
            # xdiv(g0) + transposes + o-proj(g0) hide in slots 8-15
            ot_g = [
                persist.tile([128, 1024], f32, tag=f"otg{g}", name=f"otg{g}")
                for g in range(2)
            ]

            def emit_oproj(oc, g, eng="act"):
                # quarter-granular: each 128-wide n-chunk only needs its own
                # xT transpose, so the last transpose gates 1/4 of the work
                ps0 = psum.tile([128, 1024], f32, tag="stg", name=f"op{oc}{g}")
                ps = ps0[:, 0:512]
                ot = ot_g[g][:, oc * 512 : oc * 512 + 512]
                for q in range(4):
                    for cc in range(2):
                        nc.tensor.matmul(
                            ps[:, q * 128 : q * 128 + 128],
                            lhsT=wm_sb[cc][:, oc * 128 : (oc + 1) * 128],
                            rhs=xT_sb[:, (g * 4 + q) * 256 + cc * 128 : (g * 4 + q) * 256 + cc * 128 + 128],
                            start=(cc == 0),
                            stop=(cc == 1) and not has_bm,
                        )
                    if has_bm:
                        nc.tensor.matmul(
                            ps[:, q * 128 : q * 128 + 128],
                            lhsT=bmr_sb[:, oc * 128 : (oc + 1) * 128],
                            rhs=ones1[:, 0:128],
                            start=False,
                            stop=True,
                        )
                if eng == "act":
                    nc.scalar.activation(out=ot, in_=ps, func=Act.Copy)
                else:
                    nc.vector.tensor_copy(out=ot, in_=ps)
                nc.sync.dma_start(
                    out=out_d[oc * 128 : (oc + 1) * 128, g * 512 : g * 512 + 512],
                    in_=ot,
                )

            xa0 = [psum.tile([128, 512], f32, tag="xa", bufs=4, name=f"xa0_{j}") for j in range(4)]
            xa1 = [None] * 4

            def emit_pv1_pair(mc2):
                for mc in (2 * mc2, 2 * mc2 + 1):
                    for j in range(4):
                        for h in range(H):
                            nc.tensor.matmul(
                                xa1[j][:, AUG * h : AUG * h + AUG],
                                lhsT=e_sb[mc // 2][
                                    :,
                                    2048 * (mc % 2) + 512 * h + 128 * j : 2048 * (mc % 2)
                                    + 512 * h
                                    + 128 * (j + 1),
                                ],
                                rhs=vta_sb[mc][:, AUG * h : AUG * h + AUG],
                                start=(mc == 0 and h == 0),
                                stop=(mc == MCH - 1),
                                skip_group_check=True,
                            )

            for mc in range(MCH):
                if mc < 8:
                    emit_pv_slot(xa0, 2 * mc)
                    emit_pv_slot(xa0, 2 * mc + 1)
                emit_st_slot(1, mc)
                if mc == 8:
                    emit_xdiv_j(xa0, 0, 0)
                    emit_xdiv_j(xa0, 0, 1)
                if mc == 10:
                    emit_xdiv_j(xa0, 0, 2)
                    emit_xdiv_j(xa0, 0, 3)
                if mc == 11:
                    for j in range(4):
                        xa1[j] = psum.tile(
                            [128, 512], f32, tag="xa", bufs=4, name=f"xa1_{j}"
                        )
                if mc >= 11:
                    emit_pv1_pair(mc - 11)

            # tail: last PV pairs, g0-half o-proj under them, then xdiv(g1),
            # transposes, and the g1-half o-proj
            emit_pv1_pair(5)
            emit_oproj(0, 0, eng="dve")
            emit_pv1_pair(6)
            emit_oproj(1, 0, eng="dve")
            # last pair j-outer; recip fires per j as its column finishes,
            # the divisions follow on alternating engines
            rrs = []
            for j in range(4):
                for mc in (14, 15):
                    for h in range(H):
                        nc.tensor.matmul(
                            xa1[j][:, AUG * h : AUG * h + AUG],
                            lhsT=e_sb[mc // 2][
                                :,
                                2048 * (mc % 2) + 512 * h + 128 * j : 2048 * (mc % 2)
                                + 512 * h
                                + 128 * (j + 1),
                            ],
                            rhs=vta_sb[mc][:, AUG * h : AUG * h + AUG],
                            start=False,
                            stop=(mc == MCH - 1),
                            skip_group_check=True,
                        )
                rr = work.tile([128, H], f32, tag="rr", bufs=4, name=f"rr1_{j}")
                den = (
                    xa1[j][:, HD : HD + AUG * H]
                    .rearrange("p (a b) -> p a b", b=AUG)[:, :, 0]
                )
                nc.vector.reciprocal(out=rr, in_=den)
                rrs.append(rr)
            for j in range(4):
                jj = 4 + j
                xp = xp_sb[jj // 2][:, 256 * (jj % 2) : 256 * (jj % 2) + 256]
                for h in range(H):
                    if j % 2 == 0:
                        nc.scalar.activation(
                            out=xp[:, HD * h : HD * h + HD],
                            in_=xa1[j][:, AUG * h : AUG * h + HD],
                            func=Act.Copy,
                            scale=rrs[j][:, h : h + 1],
                        )
                    else:
                        nc.vector.tensor_scalar(
                            out=xp[:, HD * h : HD * h + HD],
                            in0=xa1[j][:, AUG * h : AUG * h + HD],
                            scalar1=rrs[j][:, h : h + 1],
                            scalar2=None,
                            op0=Alu.mult,
                        )
                if jj % 2 == 1:
                    dst = xT_sb[:, (jj - 1) * 256 : (jj + 1) * 256].rearrange(
                        "p (a n) -> p a n", a=4
                    )
                    nc.sync.dma_start_transpose(out=dst, in_=xp_sb[jj // 2][:])
            emit_oproj(0, 1, eng="act")
            emit_oproj(1, 1, eng="dve")
    return nc


def _legalize_multi_waits(j):
    """This walrus build accepts at most ONE sync-wait per TPB instruction
    ("Too many sync wait commands" in setupSyncWait), but Tile emits several.
    Split: keep the last wait on the instruction and hoist the others onto
    standalone single-wait EventSemaphore ops just before it on the same
    (in-order) engine queue -- semantics preserved."""
    ctr = 0
    for f in j["functions"]:
        for b in f["blocks"]:
            out = []
            for inst in b["instructions"]:
                si = inst.get("sync_info")
                ow = (si or {}).get("on_wait") or []
                if len(ow) > 1:
                    for w in ow[:-1]:
                        ctr += 1
                        out.append(
                            {
                                "debug": inst.get("debug", 0),
                                "engine": inst["engine"],
                                "ins": [],
                                "name": f"legwait-{ctr}",
                                "opcode": "EventSemaphore",
                                "outs": [],
                                "sync_info": {"on_update": [], "on_wait": [w]},
                            }
                        )
                    si["on_wait"] = [ow[-1]]
                out.append(inst)
            b["instructions"] = out
    return j


def _get_nc(has_bq, has_bk, has_bm):
    global _NC, _NC_KEY
    key = (has_bq, has_bk, has_bm)
    if _NC is None or _NC_KEY != key:
        import json as _json
        import types as _types

        nc = _build_nc(*key)
        raw = nc.to_json_bytes()
        fixed = _json.dumps(_legalize_multi_waits(_json.loads(raw))).encode()
        nc.to_json_bytes = _types.MethodType(lambda self: fixed, nc)
        _NC = nc
        _NC_KEY = key
    return _NC


def _prep_shards(inputs):
    f = lambda k: np.asarray(inputs[k], dtype=np.float32)
    q, k, v = f("query"), f("key"), f("value")
    w, mask = f("weight"), f("mask")
    Wq, bq = f("Wq"), f("bq")
    Wk, bk = f("Wk"), f("bk")
    Wv, bv = f("Wv"), f("bv")
    Wm, bm = f("Wm"), f("bm")

    has_bq = bool(np.any(bq != 0))
    has_bk = bool(np.any(bk != 0))
    has_bv = bool(np.any(bv != 0))
    has_bm = bool(np.any(bm != 0))

    p = _PERM
    wqT = np.ascontiguousarray((Wq[p] / 8.0).T).astype(BF)
    wkT = np.ascontiguousarray(Wk[p].T).astype(BF)
    WvTp = Wv[p].T  # [in, c']
    wvT = np.zeros((D, VA), np.float32)
    bva = np.zeros((1, VA), np.float32)
    bvp = bv[p]
    for h in range(H):
        wvT[:, AUG * h : AUG * h + HD] = WvTp[:, HD * h : HD * (h + 1)]
        if has_bv:
            bva[0, AUG * h : AUG * h + HD] = bvp[HD * h : HD * (h + 1)]
        bva[0, AUG * h + HD] = 1.0
    wvT = wvT.astype(BF)
    bva = bva.astype(BF)
    wmT = np.ascontiguousarray(Wm[:, p].T).astype(BF)

    maskT = np.ascontiguousarray(mask.transpose(0, 2, 1)).astype(BF)  # [B, M, N]
    kw = k * w[:, None, :]  # weight folded into key
    qb, kb, vb = q.astype(BF), kw.astype(BF), v.astype(BF)

    in_maps = []
    for c in range(NCORES):
        b, half = c // 2, c % 2
        n0 = half * NH
        im = dict(
            xq=np.ascontiguousarray(qb[b, :, n0 : n0 + NH]),
            xk=kb[b],
            xv=vb[b],
            maskT=np.ascontiguousarray(maskT[b, :, n0 : n0 + NH]),
            wqT=wqT,
            wkT=wkT,
            wvT=wvT,
            wmT=wmT,
            bvar=bva,
        )
        if has_bq:
            im["bqr"] = np.ascontiguousarray((bq[p] / 8.0).reshape(1, D)).astype(BF)
        if has_bk:
            im["bkr"] = np.ascontiguousarray(bk[p].reshape(1, D)).astype(BF)
            im["wrow"] = np.ascontiguousarray(w[b : b + 1]).astype(BF)
        if has_bm:
            im["bmr"] = np.ascontiguousarray(bm.reshape(1, D)).astype(BF)
        in_maps.append(im)
    return in_maps, (has_bq, has_bk, has_bm)


LAST_RESULT = None  # BassKernelResults of the most recent run (for profiling)


def kernel(**inputs) -> np.ndarray:
    from concourse.bass_utils import run_bass_kernel_spmd

    in_maps, bias_key = _prep_shards(inputs)
    nc = _get_nc(*bias_key)
    global LAST_RESULT
    LAST_RESULT = run_bass_kernel_spmd(nc, in_maps, core_ids=list(range(NCORES)))
    out = np.empty((B, D, N), np.float32)
    for c in range(NCORES):
        b, half = c // 2, c % 2
        out[b, :, half * NH : (half + 1) * NH] = LAST_RESULT.results[c]["out"]
    return out


def hostsim(**inputs) -> np.ndarray:
    """Numpy re-implementation of the device pipeline (incl. bf16 casts and
    the transposed-PV + augmented-ones math) for offline validation."""
    in_maps, (has_bq, has_bk, has_bm) = _prep_shards(inputs)
    out = np.empty((B, D, N), np.float32)
    for c in range(NCORES):
        im = in_maps[c]
        b, half = c // 2, c % 2
        xq = im["xq"].astype(np.float32)
        xk = im["xk"].astype(np.float32)
        xv = im["xv"].astype(np.float32)
        wq = im["wqT"].astype(np.float32)
        wk = im["wkT"].astype(np.float32)
        wv = im["wvT"].astype(np.float32)
        wm = im["wmT"].astype(np.float32)
        mk = im["maskT"].astype(np.float32)
        q = wq.T @ xq  # [256, NH] f32 (stays f32 on device)
        if has_bq:
            q += im["bqr"].astype(np.float32).reshape(-1, 1)
        k = wk.T @ xk
        if has_bk:
            k += im["bkr"].astype(np.float32).reshape(-1, 1) * im["wrow"].astype(np.float32)
        vTa = (xv.T @ wv + im["bvar"].astype(np.float32)).astype(BF).astype(np.float32)
        x = np.empty((NH, D), np.float32)  # [n, c]
        for h in range(H):
            qh = q[64 * h : 64 * h + 64]
            kh = k[64 * h : 64 * h + 64]
            ST = kh.T @ qh  # [M, NH]
            sm = (ST * mk).astype(BF).astype(np.float32)
            E = np.exp(sm).astype(BF).astype(np.float32)
            Xa = E.T @ vTa[:, AUG * h : AUG * (h + 1)]  # [NH, 65]
            xn = Xa[:, 0:HD] / Xa[:, HD : HD + 1]
            x[:, 64 * h : 64 * h + 64] = xn.astype(BF).astype(np.float32)
        o = wm.T @ x.T.astype(np.float32)
        if has_bm:
            o += im["bmr"].astype(np.float32).reshape(-1, 1)
        out[b, :, half * NH : (half + 1) * NH] = o
    return out



# revision 45
# speedup vs baseline: 1.0294x; 1.0294x over previous
"""KMultiHeadedAttention Trainium2 kernel (v2).

Full-input contract: kernel(**inputs) takes the unsharded numpy inputs and
returns the full [4, 256, 2048] output. Internally shards (batch, N-half)
across 8 NeuronCores; each core computes QKV projections for its batch, the
attention for its 1024 query positions, and the output projection for its
slice. Output slices are disjoint -> plain concatenation, no collectives.

Per-core pipeline (b = batch, n in its half; head-major channel perm p):
  host: xk = key[b] * weight[b]   (the multiplicative key-wise weight
        commutes through the 1x1 conv: (Wk@key)*w = Wk@(key*w); a nonzero
        bk is restored as the rank-1 update bk (x) w on the PE)
  q = (Wq[p]/8) @ xq   [256, 1024]  -- PSUM -> SBUF as f32r copies split
  k = Wk[p] @ xk       [256, 2048]  -- across DVE/Act (f32r: 1 PE cyc/row)
  vTa[m, 65h+d] = xv^T @ Wv[p]^T with a ones column per head (rank-1
        ones x bva matmul), converted f32->bf16 on Act
  ST[m, n] = k_h^T q_h            f32r matmuls into [128,1024] PSUM pairs
  sm = ST * maskT                 DVE 1024-wide (the ONLY psum-reading
                                  elementwise pass; mask bf16, broadcast
                                  across the 2 heads sharing a stg tile)
  E = exp(sm)                     Act 4096-wide bf16 tiles (mc-pairs)
  Xa[n, 65h:65h+65] += E_slice^T @ vTa_slice   (PV transposed: 65-row
        output free dim halves PE rows vs the [d, n] layout). NOTE:
        matmul start=True zeroes the WHOLE psum bank, so only the first
        accumulation group per xa tile sets it.
  x[n, c] = Xa[:, :64] * recip(Xa[:, 64])  (strided recip on DVE; scaled
        Copy on Act / tensor_scalar on DVE)
  xT via SBUF->SBUF DmaTranspose (j-major layout), out = Wm[p'] @ xT

Schedule: two n-512 groups. g0 staging overlaps the v-projection; g1
staging carries PV(g0) as mc-pairs in slots 0-7, xdiv(g0) in slots 8-11,
and pre-starts PV(g1) pairs 0-4 in slots 11-15 so the tail is only the
last PV pairs + xdiv(g1) + transposes + output projection, with the tail
elementwise on whichever of DVE/Act is idle.

TimelineSim: 104.6 us/core (baseline 141.9).
"""

import sys

sys.path.insert(0, "/opt/trn_rl_repo")

import ml_dtypes
import numpy as np

B, D, N, M = 4, 256, 2048, 2048
H, HD = 4, 64
NCORES = 8
NH = N // 2  # query positions per core
AUG = HD + 1  # per-head vTa columns incl. ones column
VA = H * AUG  # 260
MCH = M // 128  # 16 m-chunks
BF = ml_dtypes.bfloat16

# new channel c' = 64h + d  <-  old channel 4d + h
_PERM = np.array([4 * d + h for h in range(H) for d in range(HD)])

_NC = None
_NC_KEY = None


def _build_nc(has_bq=False, has_bk=False, has_bm=False):
    import concourse.bass as bass
    import concourse.tile as tile
    from concourse import mybir

    f32 = mybir.dt.float32
    f32r = mybir.dt.float32r
    bf16 = mybir.dt.bfloat16
    Alu = mybir.AluOpType
    Act = mybir.ActivationFunctionType

    nc = bass.Bass()
    xq_d = nc.declare_dram_parameter("xq", [D, NH], bf16, isOutput=False)
    xk_d = nc.declare_dram_parameter("xk", [D, M], bf16, isOutput=False)
    xv_d = nc.declare_dram_parameter("xv", [D, M], bf16, isOutput=False)
    maskT_d = nc.declare_dram_parameter("maskT", [M, NH], bf16, isOutput=False)
    wq_d = nc.declare_dram_parameter("wqT", [D, D], bf16, isOutput=False)
    wk_d = nc.declare_dram_parameter("wkT", [D, D], bf16, isOutput=False)
    wv_d = nc.declare_dram_parameter("wvT", [D, VA], bf16, isOutput=False)
    wm_d = nc.declare_dram_parameter("wmT", [D, D], bf16, isOutput=False)
    bva_d = nc.declare_dram_parameter("bvar", [1, VA], bf16, isOutput=False)
    eye_d = nc.declare_dram_parameter("eye", [128, 128], bf16, isOutput=False)
    if has_bq:
        bqr_d = nc.declare_dram_parameter("bqr", [1, D], bf16, isOutput=False)
    if has_bk:
        bkr_d = nc.declare_dram_parameter("bkr", [1, D], bf16, isOutput=False)
        wrow_d = nc.declare_dram_parameter("wrow", [1, M], bf16, isOutput=False)
    if has_bm:
        bmr_d = nc.declare_dram_parameter("bmr", [1, D], bf16, isOutput=False)
    out_d = nc.declare_dram_parameter("out", [D, NH], bf16, isOutput=True)

    with tile.TileContext(nc) as tc:
        with (
            tc.tile_pool(name="consts", bufs=1) as consts,
            tc.tile_pool(name="pin", bufs=1) as pin,
            tc.tile_pool(name="persist", bufs=1) as persist,
            tc.tile_pool(name="work", bufs=3) as work,
            tc.tile_pool(name="ps", bufs=2, space="PSUM") as psum,
        ):
            # ---- weights + activations (DMAs; tile deps gate consumers) ----
            wq_sb, wk_sb, wv_sb, wm_sb = [], [], [], []
            xq_sb, xk_sb, xv_sb = [], [], []
            for i in range(2):
                t = consts.tile([128, D], bf16, tag=f"wk{i}", name=f"wk{i}")
                nc.sync.dma_start(out=t, in_=wk_d[i * 128 : (i + 1) * 128, :])
                wk_sb.append(t)
                xk_sb.append(pin.tile([128, M], bf16, tag=f"xk{i}", name=f"xk{i}"))
                nc.sync.dma_start(
                    out=xk_sb[i][:, 0:1024], in_=xk_d[i * 128 : (i + 1) * 128, 0:1024]
                )
            for i in range(2):
                t = consts.tile([128, D], bf16, tag=f"wq{i}", name=f"wq{i}")
                nc.sync.dma_start(out=t, in_=wq_d[i * 128 : (i + 1) * 128, :])
                wq_sb.append(t)
                t = pin.tile([128, NH], bf16, tag=f"xq{i}", name=f"xq{i}")
                nc.sync.dma_start(out=t, in_=xq_d[i * 128 : (i + 1) * 128, :])
                xq_sb.append(t)
            for i in range(2):
                nc.sync.dma_start(
                    out=xk_sb[i][:, 1024:2048],
                    in_=xk_d[i * 128 : (i + 1) * 128, 1024:2048],
                )
            if has_bq:
                bqr_sb = consts.tile([1, D], bf16, tag="bqr", name="bqr")
                nc.sync.dma_start(out=bqr_sb, in_=bqr_d[:])
            if has_bk:
                bkr_sb = consts.tile([1, D], bf16, tag="bkr", name="bkr")
                nc.sync.dma_start(out=bkr_sb, in_=bkr_d[:])
                wrow_sb = consts.tile([1, M], bf16, tag="wrow", name="wrow")
                nc.sync.dma_start(out=wrow_sb, in_=wrow_d[:])
            if has_bm:
                bmr_sb = consts.tile([1, D], bf16, tag="bmr", name="bmr")
                nc.sync.dma_start(out=bmr_sb, in_=bmr_d[:])
            ones1 = consts.tile([1, 512], bf16, tag="ones1", name="ones1")
            nc.vector.memset(ones1, 1.0)

            # ---- projections: k(mh0) first so STs can start early ----
            q_sb = [persist.tile([128, NH], f32r, tag=f"q{i}", name=f"qsb{i}") for i in range(2)]
            k_sb = [persist.tile([128, M], f32r, tag=f"k{i}", name=f"ksb{i}") for i in range(2)]

            def emit_kproj(oc, mh, eng):
                if mh == 0:
                    pss = [psum.tile([128, 1024], f32, tag="stg", name=f"kp{oc}{mh}")]
                    slices = [(pss[0][:, j : j + 512], j) for j in (0, 512)]
                else:
                    pss = [
                        psum.tile([128, 512], f32, tag="xa", bufs=4, name=f"kp{oc}{mh}{j}")
                        for j in (0, 512)
                    ]
                    slices = [(pss[0], 0), (pss[1], 512)]
                for ps, j in slices:
                    for ic in range(2):
                        nc.tensor.matmul(
                            ps,
                            lhsT=wk_sb[ic][:, oc * 128 : (oc + 1) * 128],
                            rhs=xk_sb[ic][:, mh * 1024 + j : mh * 1024 + j + 512],
                            start=(ic == 0),
                            stop=(ic == 1) and not has_bk,
                        )
                    if has_bk:
                        # (Wk@key + bk)*w = Wk@(key*w) + bk (x) w
                        nc.tensor.matmul(
                            ps,
                            lhsT=bkr_sb[:, oc * 128 : (oc + 1) * 128],
                            rhs=wrow_sb[:, mh * 1024 + j : mh * 1024 + j + 512],
                            start=False,
                            stop=True,
                        )
                    dst = k_sb[oc][:, mh * 1024 + j : mh * 1024 + j + 512]
                    e = eng if eng != "mix" else ("dve" if oc == 0 else "act")
                    if e == "dve":
                        nc.vector.tensor_copy(out=dst, in_=ps)
                    else:
                        nc.scalar.activation(out=dst, in_=ps, func=Act.Copy)

            for oc in range(2):
                emit_kproj(oc, 0, "mix")
            for oc in range(2):
                for j in (0, 512):
                    ps = psum.tile([128, 512], f32, tag="xa", bufs=4, name=f"qp{oc}{j}")
                    for ic in range(2):
                        nc.tensor.matmul(
                            ps,
                            lhsT=wq_sb[ic][:, oc * 128 : (oc + 1) * 128],
                            rhs=xq_sb[ic][:, j : j + 512],
                            start=(ic == 0),
                            stop=(ic == 1) and not has_bq,
                        )
                    if has_bq:
                        nc.tensor.matmul(
                            ps,
                            lhsT=bqr_sb[:, oc * 128 : (oc + 1) * 128],
                            rhs=ones1,
                            start=False,
                            stop=True,
                        )
                    dst = q_sb[oc][:, j : j + 512]
                    if oc == 0:
                        nc.scalar.activation(out=dst, in_=ps, func=Act.Copy)
                    else:
                        nc.vector.tensor_copy(out=dst, in_=ps)
            for oc in range(2):
                emit_kproj(oc, 1, "act")

            # first few mask tiles before the v/w loads (the first mults
            # need them early); the rest prefetch during staging
            msk_sb = []
            for mc in range(MCH):
                msk_sb.append(
                    persist.tile([128, NH], bf16, tag=f"mk{mc}", name=f"mk{mc}")
                )
            for mc in range(6):
                nc.sync.dma_start(
                    out=msk_sb[mc], in_=maskT_d[mc * 128 : (mc + 1) * 128, :]
                )
            for i in range(2):
                t = pin.tile([128, M], bf16, tag=f"xv{i}", name=f"xv{i}")
                nc.sync.dma_start(out=t, in_=xv_d[i * 128 : (i + 1) * 128, :])
                xv_sb.append(t)
                t = consts.tile([128, VA], bf16, tag=f"wv{i}", name=f"wv{i}")
                nc.sync.dma_start(out=t, in_=wv_d[i * 128 : (i + 1) * 128, :])
                wv_sb.append(t)
                t = consts.tile([128, D], bf16, tag=f"wm{i}", name=f"wm{i}")
                nc.sync.dma_start(out=t, in_=wm_d[i * 128 : (i + 1) * 128, :])
                wm_sb.append(t)
            bva_sb = consts.tile([1, VA], bf16, tag="bvar", name="bvar")
            nc.sync.dma_start(out=bva_sb, in_=bva_d[:])
            eye_sb = consts.tile([128, 128], bf16, tag="eye", name="eye")
            nc.sync.dma_start(out=eye_sb, in_=eye_d[:])
            for mc in range(6, MCH):
                nc.sync.dma_start(
                    out=msk_sb[mc], in_=maskT_d[mc * 128 : (mc + 1) * 128, :]
                )

            vta_sb = [
                persist.tile([128, VA], bf16, tag=f"vta{mc}", name=f"vta{mc}")
                for mc in range(MCH)
            ]
            # E tiles hold TWO m-chunks ([mc even | mc odd]) so exp runs on
            # 4096-wide tiles (fewer Act access-latency prologues)
            e_sb = [
                persist.tile([128, 4096], bf16, tag=f"e{p}", name=f"e{p}")
                for p in range(MCH // 2)
            ]
            # x in [n, c] layout: per n-chunk-of-128 a [128, 256] tile holding
            # all 4 heads' 64 channels side by side (transpose-DMA input)
            xp_sb = [
                persist.tile([128, 2 * D], bf16, tag=f"xp{jp}", name=f"xp{jp}")
                for jp in range(4)
            ]
            # xT: [c', n] j-major: xT[p, j*256 + a*128 + n] = x channel
            # a*128+p at position j*128+n  (lets one DmaTranspose cover a
            # j-PAIR and keeps the o-proj rhs a simple strided AP)
            xT_sb = persist.tile([128, 2 * NH], bf16, tag="xT", name="xT")

            sm_pair = [None]

            def emit_st_slot(g, mc):
                """ST matmuls + mask mult for slot (g, mc); exp fires on the
                4096-wide sm pair after each odd mc."""
                stg0 = psum.tile([128, 1024], f32, tag="stg", name=f"st{g}_{mc}a")
                stg1 = psum.tile([128, 1024], f32, tag="stg", name=f"st{g}_{mc}b")
                for h in range(H):
                    t = stg0 if h < 2 else stg1
                    po = 64 * (h % 2)
                    nc.tensor.matmul(
                        t[:, 512 * (h % 2) : 512 * (h % 2) + 512],
                        lhsT=k_sb[h // 2][po : po + 64, mc * 128 : (mc + 1) * 128],
                        rhs=q_sb[h // 2][po : po + 64, g * 512 : g * 512 + 512],
                        start=True,
                        stop=True,
                    )
                if mc % 2 == 0:
                    sm_pair[0] = work.tile(
                        [128, 4096], bf16, tag="sm", bufs=3, name=f"sm{g}_{mc}"
                    )
                sm = sm_pair[0]
                off = 2048 * (mc % 2)
                mk = msk_sb[mc][:, g * 512 : g * 512 + 512]
                mkb = mk.unsqueeze(1).to_broadcast([128, 2, 512])
                for hp, stg in ((0, stg0), (1, stg1)):
                    nc.vector.tensor_tensor(
                        out=sm[:, off + 1024 * hp : off + 1024 * hp + 1024],
                        in0=stg,
                        in1=mkb,
                        op=Alu.mult,
                    )
                if g == 1 and mc == 15:
                    for hp in range(2):
                        nc.scalar.activation(
                            out=e_sb[7][:, off + 1024 * hp : off + 1024 * hp + 1024],
                            in_=sm[:, off + 1024 * hp : off + 1024 * hp + 1024],
                            func=Act.Exp,
                        )
                elif g == 1 and mc == 14:
                    nc.scalar.activation(
                        out=e_sb[7][:, off : off + 2048],
                        in_=sm[:, off : off + 2048],
                        func=Act.Exp,
                    )
                elif mc % 2 == 1:
                    nc.scalar.activation(out=e_sb[mc // 2], in_=sm, func=Act.Exp)

            def emit_v_piece(mc):
                vps0 = psum.tile([128, 512], f32, tag="xa", bufs=4, name=f"vp{mc}")
                vps = vps0[:, 0:VA]
                for ic in range(2):
                    nc.tensor.matmul(
                        vps,
                        lhsT=xv_sb[ic][:, mc * 128 : (mc + 1) * 128],
                        rhs=wv_sb[ic],
                        start=(ic == 0),
                        stop=False,
                    )
                # ones column (and bv bias if nonzero) as rank-1 ones x bva
                nc.tensor.matmul(
                    vps, lhsT=ones1[:, 0:128], rhs=bva_sb, start=False, stop=True
                )
                nc.scalar.activation(out=vta_sb[mc], in_=vps, func=Act.Copy)

            def emit_pv_slot(xa, mc):
                # NOTE: start=True zeroes the WHOLE psum bank, so only the
                # first matmul of each xa tile may set it; the other head
                # groups accumulate from the zeroed bank.
                for j in range(4):
                    for h in range(H):
                        nc.tensor.matmul(
                            xa[j][:, AUG * h : AUG * h + AUG],
                            lhsT=e_sb[mc // 2][
                                :,
                                2048 * (mc % 2) + 512 * h + 128 * j : 2048 * (mc % 2)
                                + 512 * h
                                + 128 * (j + 1),
                            ],
                            rhs=vta_sb[mc][:, AUG * h : AUG * h + AUG],
                            start=(mc == 0 and h == 0),
                            stop=(mc == MCH - 1),
                            skip_group_check=True,
                        )

            def emit_xdiv_j(xa, g, j, eng="act"):
                rr = work.tile([128, H], f32, tag="rr", bufs=4, name=f"rr{g}_{j}")
                den = (
                    xa[j][:, HD : HD + AUG * H]
                    .rearrange("p (a b) -> p a b", b=AUG)[:, :, 0]
                )
                nc.vector.reciprocal(out=rr, in_=den)
                jj = 4 * g + j
                xp = xp_sb[jj // 2][:, 256 * (jj % 2) : 256 * (jj % 2) + 256]
                for h in range(H):
                    if eng == "act":
                        nc.scalar.activation(
                            out=xp[:, HD * h : HD * h + HD],
                            in_=xa[j][:, AUG * h : AUG * h + HD],
                            func=Act.Copy,
                            scale=rr[:, h : h + 1],
                        )
                    else:
                        nc.vector.tensor_scalar(
                            out=xp[:, HD * h : HD * h + HD],
                            in0=xa[j][:, AUG * h : AUG * h + HD],
                            scalar1=rr[:, h : h + 1],
                            scalar2=None,
                            op0=Alu.mult,
                        )
                if jj % 2 == 1:
                    # transpose the pair [n128, 512] -> 4 xT column blocks
                    dst = xT_sb[:, (jj - 1) * 256 : (jj + 1) * 256].rearrange(
                        "p (a n) -> p a n", a=4
                    )
                    nc.sync.dma_start_transpose(out=dst, in_=xp_sb[jj // 2][:])

            # ---- attention ----
            # g=0: staging + v-projection pieces
            for mc in range(MCH):
                emit_st_slot(0, mc)
                emit_v_piece(mc)
            # g=1: staging; PV(g=0) runs as mc-PAIRS in slots 0-7 so that
            # xdiv(g0) + transposes + o-proj(g0) hide in slots 8-15
            ot_g = [
                persist.tile([128, 1024], bf16, tag=f"otg{g}", name=f"otg{g}")
                for g in range(2)
            ]

            def emit_oproj(oc, g, eng="act"):
                # quarter-granular: each 128-wide n-chunk only needs its own
                # xT transpose, so the last transpose gates 1/4 of the work
                ps0 = psum.tile([128, 1024], f32, tag="stg", name=f"op{oc}{g}")
                ps = ps0[:, 0:512]
                ot = ot_g[g][:, oc * 512 : oc * 512 + 512]
                for q in range(4):
                    for cc in range(2):
                        nc.tensor.matmul(
                            ps[:, q * 128 : q * 128 + 128],
                            lhsT=wm_sb[cc][:, oc * 128 : (oc + 1) * 128],
                            rhs=xT_sb[:, (g * 4 + q) * 256 + cc * 128 : (g * 4 + q) * 256 + cc * 128 + 128],
                            start=(cc == 0),
                            stop=(cc == 1) and not has_bm,
                        )
                    if has_bm:
                        nc.tensor.matmul(
                            ps[:, q * 128 : q * 128 + 128],
                            lhsT=bmr_sb[:, oc * 128 : (oc + 1) * 128],
                            rhs=ones1[:, 0:128],
                            start=False,
                            stop=True,
                        )
                if eng == "act":
                    nc.scalar.activation(out=ot, in_=ps, func=Act.Copy)
                else:
                    nc.vector.tensor_copy(out=ot, in_=ps)
                if oc == 1:
                    # one merged 3D-AP DMA for both oc halves: saves a fixed
                    # HWDGE slot and the serialization between the two
                    nc.sync.dma_start(
                        out=out_d[:, g * 512 : g * 512 + 512].rearrange(
                            "(o p) c -> p o c", o=2
                        ),
                        in_=ot_g[g].rearrange("p (o c) -> p o c", c=512),
                    )

            xa0 = [psum.tile([128, 512], f32, tag="xa", bufs=4, name=f"xa0_{j}") for j in range(4)]
            xa1 = [None] * 4

            def emit_pv1_pair(mc2):
                for mc in (2 * mc2, 2 * mc2 + 1):
                    for j in range(4):
                        for h in range(H):
                            nc.tensor.matmul(
                                xa1[j][:, AUG * h : AUG * h + AUG],
                                lhsT=e_sb[mc // 2][
                                    :,
                                    2048 * (mc % 2) + 512 * h + 128 * j : 2048 * (mc % 2)
                                    + 512 * h
                                    + 128 * (j + 1),
                                ],
                                rhs=vta_sb[mc][:, AUG * h : AUG * h + AUG],
                                start=(mc == 0 and h == 0),
                                stop=(mc == MCH - 1),
                                skip_group_check=True,
                            )

            for mc in range(MCH):
                if mc < 8:
                    emit_pv_slot(xa0, 2 * mc)
                    emit_pv_slot(xa0, 2 * mc + 1)
                emit_st_slot(1, mc)
                if mc == 8:
                    emit_xdiv_j(xa0, 0, 0)
                    emit_xdiv_j(xa0, 0, 1)
                if mc == 10:
                    emit_xdiv_j(xa0, 0, 2)
                    emit_xdiv_j(xa0, 0, 3)
                if mc == 11:
                    for j in range(4):
                        xa1[j] = psum.tile(
                            [128, 512], f32, tag="xa", bufs=4, name=f"xa1_{j}"
                        )
                if mc >= 11:
                    emit_pv1_pair(mc - 11)

            # tail: last PV pairs, g0-half o-proj under them, then xdiv(g1),
            # transposes, and the g1-half o-proj
            emit_pv1_pair(5)
            emit_oproj(0, 0, eng="dve")
            emit_pv1_pair(6)
            emit_oproj(1, 0, eng="dve")
            # last pair j-outer; recip fires per j as its column finishes,
            # the divisions follow on alternating engines
            rrs = []
            for j in range(4):
                for mc in (14, 15):
                    for h in range(H):
                        nc.tensor.matmul(
                            xa1[j][:, AUG * h : AUG * h + AUG],
                            lhsT=e_sb[mc // 2][
                                :,
                                2048 * (mc % 2) + 512 * h + 128 * j : 2048 * (mc % 2)
                                + 512 * h
                                + 128 * (j + 1),
                            ],
                            rhs=vta_sb[mc][:, AUG * h : AUG * h + AUG],
                            start=False,
                            stop=(mc == MCH - 1),
                            skip_group_check=True,
                        )
                rr = work.tile([128, H], f32, tag="rr", bufs=4, name=f"rr1_{j}")
                den = (
                    xa1[j][:, HD : HD + AUG * H]
                    .rearrange("p (a b) -> p a b", b=AUG)[:, :, 0]
                )
                nc.vector.reciprocal(out=rr, in_=den)
                rrs.append(rr)
            for j in range(4):
                jj = 4 + j
                xp = xp_sb[jj // 2][:, 256 * (jj % 2) : 256 * (jj % 2) + 256]
                for h in range(H):
                    if j % 2 == 0:
                        nc.scalar.activation(
                            out=xp[:, HD * h : HD * h + HD],
                            in_=xa1[j][:, AUG * h : AUG * h + HD],
                            func=Act.Copy,
                            scale=rrs[j][:, h : h + 1],
                        )
                    else:
                        nc.vector.tensor_scalar(
                            out=xp[:, HD * h : HD * h + HD],
                            in0=xa1[j][:, AUG * h : AUG * h + HD],
                            scalar1=rrs[j][:, h : h + 1],
                            scalar2=None,
                            op0=Alu.mult,
                        )
                if jj % 2 == 1:
                    # PE block-transposes: lower latency than DmaTranspose
                    # (no HWDGE/DGE fixed costs) and keeps the PE p-state warm
                    for a in range(4):
                        tp = psum.tile(
                            [128, 128], bf16, tag="xa", bufs=4, name=f"tp{jj}_{a}"
                        )
                        nc.tensor.matmul(
                            tp,
                            lhsT=xp_sb[jj // 2][:, 128 * a : 128 * (a + 1)],
                            rhs=eye_sb,
                            is_transpose=True,
                            start=True,
                            stop=True,
                        )
                        dst = xT_sb[:, (jj - 1) * 256 + 128 * a : (jj - 1) * 256 + 128 * (a + 1)]
                        if a % 2 == 0:
                            nc.scalar.activation(out=dst, in_=tp, func=Act.Copy)
                        else:
                            nc.vector.tensor_copy(out=dst, in_=tp)
            emit_oproj(0, 1, eng="act")
            emit_oproj(1, 1, eng="dve")
    return nc


def _legalize_multi_waits(j):
    """This walrus build accepts at most ONE sync-wait per TPB instruction
    ("Too many sync wait commands" in setupSyncWait), but Tile emits several.
    Split: keep the last wait on the instruction and hoist the others onto
    standalone single-wait EventSemaphore ops just before it on the same
    (in-order) engine queue -- semantics preserved."""
    ctr = 0
    for f in j["functions"]:
        for b in f["blocks"]:
            out = []
            for inst in b["instructions"]:
                si = inst.get("sync_info")
                ow = (si or {}).get("on_wait") or []
                if len(ow) > 1:
                    for w in ow[:-1]:
                        ctr += 1
                        out.append(
                            {
                                "debug": inst.get("debug", 0),
                                "engine": inst["engine"],
                                "ins": [],
                                "name": f"legwait-{ctr}",
                                "opcode": "EventSemaphore",
                                "outs": [],
                                "sync_info": {"on_update": [], "on_wait": [w]},
                            }
                        )
                    si["on_wait"] = [ow[-1]]
                out.append(inst)
            b["instructions"] = out
    return j


def _get_nc(has_bq, has_bk, has_bm):
    global _NC, _NC_KEY
    key = (has_bq, has_bk, has_bm)
    if _NC is None or _NC_KEY != key:
        import json as _json
        import types as _types

        nc = _build_nc(*key)
        raw = nc.to_json_bytes()
        fixed = _json.dumps(_legalize_multi_waits(_json.loads(raw))).encode()
        nc.to_json_bytes = _types.MethodType(lambda self: fixed, nc)
        _NC = nc
        _NC_KEY = key
    return _NC


def _prep_shards(inputs):
    f = lambda k: np.asarray(inputs[k], dtype=np.float32)
    q, k, v = f("query"), f("key"), f("value")
    w, mask = f("weight"), f("mask")
    Wq, bq = f("Wq"), f("bq")
    Wk, bk = f("Wk"), f("bk")
    Wv, bv = f("Wv"), f("bv")
    Wm, bm = f("Wm"), f("bm")

    has_bq = bool(np.any(bq != 0))
    has_bk = bool(np.any(bk != 0))
    has_bv = bool(np.any(bv != 0))
    has_bm = bool(np.any(bm != 0))

    p = _PERM
    wqT = np.ascontiguousarray((Wq[p] / 8.0).T).astype(BF)
    wkT = np.ascontiguousarray(Wk[p].T).astype(BF)
    WvTp = Wv[p].T  # [in, c']
    wvT = np.zeros((D, VA), np.float32)
    bva = np.zeros((1, VA), np.float32)
    bvp = bv[p]
    for h in range(H):
        wvT[:, AUG * h : AUG * h + HD] = WvTp[:, HD * h : HD * (h + 1)]
        if has_bv:
            bva[0, AUG * h : AUG * h + HD] = bvp[HD * h : HD * (h + 1)]
        bva[0, AUG * h + HD] = 1.0
    wvT = wvT.astype(BF)
    bva = bva.astype(BF)
    wmT = np.ascontiguousarray(Wm[:, p].T).astype(BF)

    maskT = np.ascontiguousarray(mask.transpose(0, 2, 1)).astype(BF)  # [B, M, N]
    kw = k * w[:, None, :]  # weight folded into key
    qb, kb, vb = q.astype(BF), kw.astype(BF), v.astype(BF)

    in_maps = []
    for c in range(NCORES):
        b, half = c // 2, c % 2
        n0 = half * NH
        im = dict(
            xq=np.ascontiguousarray(qb[b, :, n0 : n0 + NH]),
            xk=kb[b],
            xv=vb[b],
            maskT=np.ascontiguousarray(maskT[b, :, n0 : n0 + NH]),
            wqT=wqT,
            wkT=wkT,
            wvT=wvT,
            wmT=wmT,
            bvar=bva,
            eye=np.eye(128, dtype=np.float32).astype(BF),
        )
        if has_bq:
            im["bqr"] = np.ascontiguousarray((bq[p] / 8.0).reshape(1, D)).astype(BF)
        if has_bk:
            im["bkr"] = np.ascontiguousarray(bk[p].reshape(1, D)).astype(BF)
            im["wrow"] = np.ascontiguousarray(w[b : b + 1]).astype(BF)
        if has_bm:
            im["bmr"] = np.ascontiguousarray(bm.reshape(1, D)).astype(BF)
        in_maps.append(im)
    return in_maps, (has_bq, has_bk, has_bm)


LAST_RESULT = None  # BassKernelResults of the most recent run (for profiling)


def kernel(**inputs) -> np.ndarray:
    from concourse.bass_utils import run_bass_kernel_spmd

    in_maps, bias_key = _prep_shards(inputs)
    nc = _get_nc(*bias_key)
    global LAST_RESULT
    LAST_RESULT = run_bass_kernel_spmd(nc, in_maps, core_ids=list(range(NCORES)))
    out = np.empty((B, D, N), np.float32)
    for c in range(NCORES):
        b, half = c // 2, c % 2
        out[b, :, half * NH : (half + 1) * NH] = np.asarray(
            LAST_RESULT.results[c]["out"]
        ).astype(np.float32)
    return out


def hostsim(**inputs) -> np.ndarray:
    """Numpy re-implementation of the device pipeline (incl. bf16 casts and
    the transposed-PV + augmented-ones math) for offline validation."""
    in_maps, (has_bq, has_bk, has_bm) = _prep_shards(inputs)
    out = np.empty((B, D, N), np.float32)
    for c in range(NCORES):
        im = in_maps[c]
        b, half = c // 2, c % 2
        xq = im["xq"].astype(np.float32)
        xk = im["xk"].astype(np.float32)
        xv = im["xv"].astype(np.float32)
        wq = im["wqT"].astype(np.float32)
        wk = im["wkT"].astype(np.float32)
        wv = im["wvT"].astype(np.float32)
        wm = im["wmT"].astype(np.float32)
        mk = im["maskT"].astype(np.float32)
        q = wq.T @ xq  # [256, NH] f32 (stays f32 on device)
        if has_bq:
            q += im["bqr"].astype(np.float32).reshape(-1, 1)
        k = wk.T @ xk
        if has_bk:
            k += im["bkr"].astype(np.float32).reshape(-1, 1) * im["wrow"].astype(np.float32)
        vTa = (xv.T @ wv + im["bvar"].astype(np.float32)).astype(BF).astype(np.float32)
        x = np.empty((NH, D), np.float32)  # [n, c]
        for h in range(H):
            qh = q[64 * h : 64 * h + 64]
            kh = k[64 * h : 64 * h + 64]
            ST = kh.T @ qh  # [M, NH]
            sm = (ST * mk).astype(BF).astype(np.float32)
            E = np.exp(sm).astype(BF).astype(np.float32)
            Xa = E.T @ vTa[:, AUG * h : AUG * (h + 1)]  # [NH, 65]
            xn = Xa[:, 0:HD] / Xa[:, HD : HD + 1]
            x[:, 64 * h : 64 * h + 64] = xn.astype(BF).astype(np.float32)
        o = wm.T @ x.T.astype(np.float32)
        if has_bm:
            o += im["bmr"].astype(np.float32).reshape(-1, 1)
        out[b, :, half * NH : (half + 1) * NH] = o.astype(BF).astype(np.float32)
    return out

